# revision 7
# baseline (speedup 1.0000x reference)
"""Trainium2 Bass kernel for a quantized Mistral-style SwiGLU MLP.

Reference computation (per token x of dim HIDDEN=4096):
    g = x @ (gate_wq * gate_scale[:, None]).T      # [INTER]
    u = x @ (up_wq   * up_scale[:, None]).T        # [INTER]
    h = silu(g) * u
    y = h @ (down_wq * down_scale[:, None]).T      # [HIDDEN]

Sharding across 8 NeuronCores: DP4 (token groups of 2048) x TP2 (intermediate
shards of 7168).  Each core runs the same SPMD program:
  phase 1: xT resident in SBUF (feature-major [hid, tok]); stream gate/up
           weight tiles; PE matmuls accumulate g,u in PSUM; ACT applies
           silu(gate_scale*g); DVE forms h = (up_scale*u) * silu(...) in bf16;
           h goes to a DRAM bounce buffer.
  phase 2: h streamed back per k-quarter (resident in SBUF); PE matmuls
           against down weight tiles accumulate each quarter's [hid, tok]
           fp32 partial in PSUM, and the 4 partials are summed in DRAM via
           SWDGE accumulate-DMA (output buffers are zero-initialized by the
           SPMD runner).
Host sums the TP pair, applies down_scale, and re-assembles [B, S, HIDDEN].

All weights are fed to the device as exact bf16 integers (values in
[-128,127] are exactly representable); scales stay fp32 and are applied
per-partition on chip (gate/up) or on host (down).
"""

import numpy as np
import ml_dtypes

import concourse.bacc as bacc
import concourse.mybir as mybir
import concourse.tile as tile
from concourse.bass_utils import run_bass_kernel_spmd

BF16 = ml_dtypes.bfloat16
BF = mybir.dt.bfloat16
F32 = mybir.dt.float32

N_CORES = 8
DP, TP = 4, 2
HIDDEN, INTER = 4096, 14336
B, S = 4, 2048

P = 128
FD = 512  # matmul moving free dim (one PSUM bank of fp32)


def dedupe_ldw(nc):
    """Drop PE InstLdweights identical to the previous one when only
    matmuls sit in between (the PE array still holds those weights).
    Only sync-free LDWs are dropped, so semaphore behavior is unchanged."""
    n_drop = 0
    for fn in nc.m.functions:
        for blk in fn.blocks:
            last_key = None
            keep = []
            for inst in blk.instructions:
                if isinstance(inst, mybir.InstLdweights):
                    key = str(inst.ins[0])
                    si = inst.sync_info
                    clean = si is None or (not si.on_wait and not si.on_update)
                    if key == last_key and clean:
                        n_drop += 1
                        continue
                    last_key = key
                elif isinstance(inst, mybir.InstMatmult):
                    pass  # consumes, does not clobber, loaded weights
                elif inst.engine == mybir.EngineType.PE:
                    last_key = None
                keep.append(inst)
            if len(keep) != len(blk.instructions):
                blk.instructions[:] = keep
    return n_drop


def slim_pe_sems(nc):
    """Drop sem-incs from matmuls nobody waits on.

    Tile gives every matmul a +1 update on the PE semaphore, but consumers
    only wait at accumulation-group boundaries (240 distinct wait values vs
    21504 incs here).  Each EVT_SEM write costs ~26ns of PE issue time, so
    keep only the awaited incs (plus the final one) and renumber the waits.
    Safe because PE completes matmuls in program order.
    """
    fn = nc.m.functions[0]
    blocks = list(fn.blocks)
    upd_by = {}
    for blk in blocks:
        for inst in blk.instructions:
            si = inst.sync_info
            if si is None:
                continue
            for u in si.on_update:
                upd_by.setdefault(u.id, set()).add(
                    (type(inst).__name__, u.update_mode, u.update_value))
    cand = {sid for sid, kinds in upd_by.items()
            if kinds == {("InstMatmult", "sem-inc", 1)}}
    n_drop = 0
    for sid in cand:
        incs, waits = [], []
        ok = True
        for blk in blocks:
            for inst in blk.instructions:
                si = inst.sync_info
                if si is None:
                    continue
                for u in si.on_update:
                    if u.id == sid:
                        incs.append(si)
                for w in si.on_wait:
                    if w.id == sid:
                        if w.wait_mode != "sem-ge-imm" or w.wait_reg is not None:
                            ok = False
                        waits.append(w)
        awaited = sorted({w.wait_value for w in waits})
        if not ok or not incs or (awaited and awaited[-1] > len(incs)):
            continue
        keep = set(awaited)
        keep.add(len(incs))
        newval = {v: i + 1 for i, v in enumerate(sorted(keep))}
        for idx, si in enumerate(incs, start=1):
            if idx not in keep:
                si.on_update = [u for u in si.on_update if u.id != sid]
                n_drop += 1
        for w in waits:
            w.wait_value = newval[w.wait_value]
    return n_drop


def build_module(hidden, inter_sh, m, kq_splits=None):
    """Build the per-core SPMD Bass module.

    hidden:   full hidden dim (contraction of phase 1, output of phase 2)
    inter_sh: this core's intermediate-dim shard
    m:        tokens per core
    """
    KH = hidden // P        # phase-1 contraction chunks
    NO = inter_sh // P      # phase-1 output tiles (inter)
    OH = hidden // P        # phase-2 output tiles (hid)
    MQ = m // FD            # moving passes per psum row
    KQ = 4 if NO % 4 == 0 else 1   # phase-2 k-quarters (h resident per quarter)
    KK = NO // KQ
    assert m % FD == 0

    nc = bacc.Bacc("TRN2", target_bir_lowering=False, debug=False,
                   num_devices=N_CORES)

    xT_d = nc.dram_tensor("xT", [P, KH, m], BF, kind="ExternalInput").ap()
    gw_d = nc.dram_tensor("gw", [NO, P, KH * P], BF, kind="ExternalInput").ap()
    uw_d = nc.dram_tensor("uw", [NO, P, KH * P], BF, kind="ExternalInput").ap()
    dw_d = nc.dram_tensor("dw", [OH, P, NO * P], BF, kind="ExternalInput").ap()
    gs_d = nc.dram_tensor("gs", [P, NO], F32, kind="ExternalInput").ap()
    us_d = nc.dram_tensor("us", [P, NO], F32, kind="ExternalInput").ap()
    y_d = nc.dram_tensor("y", [OH, P, m], F32, kind="ExternalOutput").ap()

    mult = mybir.AluOpType.mult
    silu = mybir.ActivationFunctionType.Silu

    with tile.TileContext(nc) as tc:
        with tc.tile_pool(name="const", bufs=1) as cpool, \
             tc.tile_pool(name="dram", bufs=1, space="DRAM") as dpool:
            gs_sb = cpool.tile([P, NO], F32, tag="gs")
            us_sb = cpool.tile([P, NO], F32, tag="us")
            nc.sync.dma_start(out=gs_sb[:], in_=gs_d[:])
            nc.sync.dma_start(out=us_sb[:], in_=us_d[:])
            h_d = dpool.tile([NO, P, m], BF)

            # ---------------- phase 1: h = silu(gs*g) * (us*u) ----------
            with tc.tile_pool(name="xp", bufs=1) as xp, \
                 tc.tile_pool(name="wp", bufs=2) as wp, \
                 tc.tile_pool(name="sp", bufs=2) as sp, \
                 tc.tile_pool(name="pp", bufs=1, space="PSUM") as pp:
                x_sb = xp.tile([P, KH, m], BF, tag="x")
                nc.sync.dma_start(out=x_sb[:], in_=xT_d[:])
                for o in range(NO):
                    gwt = wp.tile([P, KH * P], BF, tag="gw")
                    uwt = wp.tile([P, KH * P], BF, tag="uw")
                    nc.sync.dma_start(out=gwt[:], in_=gw_d[o])
                    nc.sync.dma_start(out=uwt[:], in_=uw_d[o])
                    pg = pp.tile([P, m], F32, tag="pg")
                    pu = pp.tile([P, m], F32, tag="pu")
                    # k-outer: each weight tile is held across the 4 moving
                    # passes, and dedupe_ldw() drops the 3 redundant
                    # LDWEIGHTS per group (measured ~30ns/MM cheaper than
                    # reloading every matmul).
                    for k in range(KH):
                        for q in range(MQ):
                            nc.tensor.matmul(
                                pg[:, q * FD:(q + 1) * FD],
                                gwt[:, k * P:(k + 1) * P],
                                x_sb[:, k, q * FD:(q + 1) * FD],
                                start=(k == 0), stop=(k == KH - 1))
                    for k in range(KH):
                        for q in range(MQ):
                            nc.tensor.matmul(
                                pu[:, q * FD:(q + 1) * FD],
                                uwt[:, k * P:(k + 1) * P],
                                x_sb[:, k, q * FD:(q + 1) * FD],
                                start=(k == 0), stop=(k == KH - 1))
                    sg = sp.tile([P, m], BF, tag="sg")
                    nc.scalar.activation(sg[:], pg[:], silu,
                                         scale=gs_sb[:, o:o + 1])
                    hb = sp.tile([P, m], BF, tag="hb")
                    nc.vector.scalar_tensor_tensor(
                        hb[:], pu[:], us_sb[:, o:o + 1], sg[:], mult, mult)
                    nc.sync.dma_start(out=h_d[o], in_=hb[:])

            # ---- phase 2: y += h[kq] @ down[kq], DMA-accumulated over kq ----
            with tc.tile_pool(name="hqp", bufs=2) as hqp, \
                 tc.tile_pool(name="dwp", bufs=2) as dwp, \
                 tc.tile_pool(name="yop", bufs=2) as yop, \
                 tc.tile_pool(name="pp2", bufs=2, space="PSUM") as pp2:
                for kq in range(KQ):
                    hq = hqp.tile([P, KK, m], BF, tag="hq")
                    for kk in range(KK):
                        nc.sync.dma_start(out=hq[:, kk, :],
                                          in_=h_d[kq * KK + kk])
                    for o in range(OH):
                        dwt = dwp.tile([P, KK * P], BF, tag="dw")
                        nc.sync.dma_start(
                            out=dwt[:],
                            in_=dw_d[o][:, kq * KK * P:(kq + 1) * KK * P])
                        py = pp2.tile([P, m], F32, tag="py")
                        for kk in range(KK):
                            for q in range(MQ):
                                nc.tensor.matmul(
                                    py[:, q * FD:(q + 1) * FD],
                                    dwt[:, kk * P:(kk + 1) * P],
                                    hq[:, kk, q * FD:(q + 1) * FD],
                                    start=(kk == 0), stop=(kk == KK - 1))
                        yo = yop.tile([P, m], F32, tag="yo")
                        nc.vector.tensor_copy(yo[:], py[:])
                        if KQ == 1:
                            nc.sync.dma_start(out=y_d[o], in_=yo[:])
                        else:
                            # accumulate partials straight into DRAM (SWDGE);
                            # output buffers are zero-initialized by the runner
                            nc.gpsimd.dma_start(out=y_d[o], in_=yo[:],
                                                accum_op=mybir.AluOpType.add)

    dedupe_ldw(nc)
    slim_pe_sems(nc)
    nc.compile()
    return nc


def prep_core_inputs(x_flat, gate_wq, gate_scale, up_wq, up_scale, down_wq,
                     hidden, inter, dp, tp, kq_splits=None):
    """Shard + repack full inputs into per-core input maps (list of dicts)."""
    n_tok = x_flat.shape[0]
    m = n_tok // dp
    inter_sh = inter // tp
    KH = hidden // P
    NO = inter_sh // P
    OH = hidden // P

    # per-TP-shard weight packs (shared by all DP groups)
    packs = []
    for s in range(tp):
        lo, hi = s * inter_sh, (s + 1) * inter_sh
        gq = gate_wq[lo:hi].astype(BF16)
        uq = up_wq[lo:hi].astype(BF16)
        # [o,c,k,p] -> [o,p,k,c] -> [NO, P, KH*P]
        gw = np.ascontiguousarray(
            gq.reshape(NO, P, KH, P).transpose(0, 3, 2, 1)).reshape(NO, P, KH * P)
        uw = np.ascontiguousarray(
            uq.reshape(NO, P, KH, P).transpose(0, 3, 2, 1)).reshape(NO, P, KH * P)
        dq = down_wq[:, lo:hi].astype(BF16)
        # [o,c,j,p] -> [o,p,j,c] -> [OH, P, NO*P]
        dw = np.ascontiguousarray(
            dq.reshape(OH, P, NO, P).transpose(0, 3, 2, 1)).reshape(OH, P, NO * P)
        gs = np.ascontiguousarray(gate_scale[lo:hi].reshape(NO, P).T)
        us = np.ascontiguousarray(up_scale[lo:hi].reshape(NO, P).T)
        packs.append(dict(gw=gw, uw=uw, dw=dw, gs=gs, us=us))

    in_maps = []
    for g in range(dp):
        xg = x_flat[g * m:(g + 1) * m]  # [m, hidden]
        xT = np.ascontiguousarray(xg.T.astype(BF16)).reshape(P * KH, m)
        # [hidden, m] with hidden = k*P + p -> [P, KH, m]
        xT = np.ascontiguousarray(
            xT.reshape(KH, P, m).transpose(1, 0, 2))
        for s in range(tp):
            in_maps.append({"xT": xT, **packs[s]})
    return in_maps


_NC_CACHE = {}


def _get_module():
    key = "full"
    if key not in _NC_CACHE:
        _NC_CACHE[key] = build_module(HIDDEN, INTER // TP, (B * S) // DP)
    return _NC_CACHE[key]


def kernel(x, gate_wq, gate_scale, up_wq, up_scale, down_wq, down_scale,
           _return_results=False):
    x = np.asarray(x)
    x_flat = x.reshape(B * S, HIDDEN)
    in_maps = prep_core_inputs(
        x_flat, np.asarray(gate_wq), np.asarray(gate_scale),
        np.asarray(up_wq), np.asarray(up_scale), np.asarray(down_wq),
        HIDDEN, INTER, DP, TP)

    nc = _get_module()
    res = run_bass_kernel_spmd(nc, in_maps, list(range(N_CORES)))

    m = (B * S) // DP
    y = np.empty((B * S, HIDDEN), np.float32)
    ds = np.asarray(down_scale).astype(np.float32)
    for g in range(DP):
        acc = None
        for s in range(TP):
            part = res.results[g * TP + s]["y"]  # [OH, P, m]
            acc = part if acc is None else acc + part
        # [OH, P, m] -> [hidden, m] -> [m, hidden]
        y[g * m:(g + 1) * m] = acc.reshape(HIDDEN, m).T
    y *= ds[None, :]
    out = y.reshape(B, S, HIDDEN)
    if _return_results:
        return out, res
    return out

